# revision 20
# baseline (speedup 1.0000x reference)
"""Trainium2 Bass kernel for the GNN message-passing decoder.

Model (per batch b):
  h0 = x @ W_lin + b_lin            -> [N=256, L2=64] per b
  h  = h0 @ W_in + b_in             -> [N, H=32]
  3 rounds of fully-connected message passing:
    rcv = h @ We1[:H], snd = h @ We1[H:]
    e1  = lrelu(rcv_i + snd_j + be1)          [N,N,HE=128]
    e2  = lrelu(e1 @ We2 + be2)               [N,N,EO=64]
    m_i = sum_j e2                            [N,EO]
    n   = lrelu([h|m] @ Wn0 + bn0); h = lrelu(n @ Wn1 + bn1)
  out = tanh(h)                      -> [B, N, H]

Strategy: pure data parallel over batch (16 -> 2 per core on 8 cores).
Per (b, round): feature-on-partition layout. For each receiver i the DVE
computes M_i = max(sndT, -b_i) in fp16 (4x mode) with a free-dim running sum
(accum) in the same instruction; since relu(z) = M_i + b_i and
lrelu(z) = 0.2 z + 0.8 relu(z), the PE accumulates
P = 0.8 We2^T M_i + 0.2 We2^T sndT into PSUM (two receivers partition-stacked
per half-bank) and the remaining per-receiver constant We2^T b_i + be2 becomes
the per-partition bias of the ACT layer-2 relu, whose accum output yields
sum_j relu(g) directly. The linear halves of both lrelus are restored exactly
with per-(b,t) batched corrections (Q = We2^T sum_j M, ssum terms).
"""

import os
import sys

import numpy as np

for _p in ("/opt/trn_rl_repo", "/opt/pypackages"):
    if _p not in sys.path and os.path.isdir(_p):
        sys.path.append(_p)

# Problem dims (hardcoded per spec)
B, N, L, H, HE, EO = 16, 256, 64, 32, 128, 64
NT = 3           # message passing rounds
NCORES = 8
BPC = B // NCORES  # batches per core = 2
NP2 = N // 2     # 128 n-pair matmuls in stage 1
SG = 3           # psum banks per supergroup (LDW amortization)

# column permutation of the node axis used on-chip (even nodes first)
PERM = np.concatenate([np.arange(0, N, 2), np.arange(1, N, 2)])
INVPERM = np.argsort(PERM)

_CACHE = {}


def _build_nc():
    import concourse.bass as bass
    import concourse.tile as tile
    from concourse import bacc, mybir
    from contextlib import ExitStack

    F16 = mybir.dt.float16
    F32 = mybir.dt.float32
    F32R = mybir.dt.float32r
    AF = mybir.ActivationFunctionType
    ALU = mybir.AluOpType

    nc = bacc.Bacc("TRN2", target_bir_lowering=False, debug=False)

    # ---- kernel I/O (per-core) ----
    xT_d = nc.dram_tensor("xT", [L, BPC], F32, kind="ExternalInput")
    wlin_d = nc.dram_tensor("Wlin", [L, N * L], F32, kind="ExternalInput")
    blT_d = nc.dram_tensor("blT", [L, N], F32, kind="ExternalInput")  # perm'd
    win_d = nc.dram_tensor("Win", [L, H], F32, kind="ExternalInput")
    binc_d = nc.dram_tensor("binc", [H, 1], F32, kind="ExternalInput")
    we1a_d = nc.dram_tensor("We1a", [H, NT * HE], F32, kind="ExternalInput")
    we1b_d = nc.dram_tensor("We1b", [H, NT * HE], F32, kind="ExternalInput")
    be1_d = nc.dram_tensor("be1c", [HE, NT], F32, kind="ExternalInput")
    we2_d = nc.dram_tensor("We2c", [HE, NT * EO], F32, kind="ExternalInput")
    be2_d = nc.dram_tensor("be2c", [EO, NT], F32, kind="ExternalInput")
    wn0_d = nc.dram_tensor("Wn0c", [H + EO, NT * H], F32, kind="ExternalInput")
    bn0_d = nc.dram_tensor("bn0c", [H, NT], F32, kind="ExternalInput")
    wn1_d = nc.dram_tensor("Wn1c", [H, NT * H], F32, kind="ExternalInput")
    bn1_d = nc.dram_tensor("bn1c", [H, NT], F32, kind="ExternalInput")
    out_d = nc.dram_tensor("out", [BPC, H, N], F32, kind="ExternalOutput")

    with tile.TileContext(nc) as tc, ExitStack() as ctx:
        const = ctx.enter_context(tc.tile_pool(name="const", bufs=1))
        perb = ctx.enter_context(tc.tile_pool(name="perb", bufs=2))
        mpool = ctx.enter_context(tc.tile_pool(name="m", bufs=6))
        small = ctx.enter_context(tc.tile_pool(name="small", bufs=4))
        psum = ctx.enter_context(tc.tile_pool(name="psum", bufs=6, space="PSUM"))
        ppsum = ctx.enter_context(tc.tile_pool(name="ppsum", bufs=2, space="PSUM"))

        # ---- load constants ----
        def load(dram, shape):
            t = const.tile(shape, F32, tag=dram.name)
            nc.sync.dma_start(t[:, :], dram[:, :])
            return t

        xTs = load(xT_d, [L, BPC])
        blT = load(blT_d, [L, N])
        win = load(win_d, [L, H])
        binc = load(binc_d, [H, 1])
        we1a = load(we1a_d, [H, NT * HE])
        we1b = load(we1b_d, [H, NT * HE])
        be1 = load(be1_d, [HE, NT])
        we2 = load(we2_d, [HE, NT * EO])
        be2 = load(be2_d, [EO, NT])
        wn0 = load(wn0_d, [H + EO, NT * H])
        bn0 = load(bn0_d, [H, NT])
        wn1 = load(wn1_d, [H, NT * H])
        bn1 = load(bn1_d, [H, NT])

        wlin = const.tile([L, N * L], F32, tag="wlin")
        for k in range(4):
            sl = bass.ts(k, N * L // 4)
            eng = nc.sync if k % 2 == 0 else nc.gpsimd
            eng.dma_start(wlin[:, sl], wlin_d[:, sl])

        # fp16 / scaled weight variants per round
        w8 = []    # 0.8 * We2, fp16 [HE, EO]
        wdd = []   # 0.2 * We2 duplicated, fp16 [HE, 2*EO]
        be2_512 = []  # 51.2 * be2 col
        for t in range(NT):
            w2sl = we2[:, bass.ts(t, EO)]
            a = const.tile([HE, EO], F16, tag=f"w8_{t}")
            nc.scalar.mul(a[:, :], w2sl, 0.8)
            w8.append(a)
            d = const.tile([HE, 2 * EO], F16, tag=f"wdd_{t}")
            nc.scalar.mul(d[:, 0:EO], w2sl, 0.2)
            nc.scalar.mul(d[:, EO:2 * EO], w2sl, 0.2)
            wdd.append(d)
            bb = const.tile([EO, 1], F32, tag=f"be2512_{t}")
            nc.scalar.mul(bb[:, :], be2[:, t:t + 1], 0.2 * N)
            be2_512.append(bb)

        # ---- stage 1: h0 = x @ W_lin in transposed (perm'd) layout ----
        h0p = ppsum.tile([128, 2 * NP2], F32, tag="prep")
        for np_ in range(NP2):
            lhsT = wlin[:, np_ * 2 * L:(np_ + 1) * 2 * L]
            nc.tensor.matmul(h0p[:, 2 * np_:2 * np_ + 2], lhsT,
                             xTs[:, :],
                             start=True, stop=True, skip_group_check=True)
        hstA = const.tile([L, 2 * NP2], F32, tag="hstA")  # even nodes
        hstB = const.tile([L, 2 * NP2], F32, tag="hstB")  # odd nodes
        nc.scalar.copy(hstA[:, :], h0p[0:L, :])
        nc.scalar.copy(hstB[:, :], h0p[L:2 * L, :])
        # columns are (np, b) interleaved; view as [p, b, np] to slice per-b
        hsvA = hstA[:, :].rearrange("p (n two) -> p two n", two=2)
        hsvB = hstB[:, :].rearrange("p (n two) -> p two n", two=2)

        hT = []  # per-b current hidden state tiles [H, N] f32
        for b in range(BPC):
            htp = ppsum.tile([H, N], F32, tag="prep")
            # full-width start=True first; partition/free-subset accumulates after
            nc.tensor.matmul(htp[:, :], win[:, :],
                             blT[:, :],
                             start=True, stop=False, skip_group_check=True)
            nc.tensor.matmul(htp[:, 0:NP2], win[:, :],
                             hsvA[:, b:b + 1, :],
                             start=False, stop=True, skip_group_check=True)
            nc.tensor.matmul(htp[:, NP2:N], win[:, :],
                             hsvB[:, b:b + 1, :],
                             start=False, stop=True, skip_group_check=True)
            ht = perb.tile([H, N], F32, tag=f"hT{b}")
            nc.scalar.activation(ht[:, :], htp[:, :], AF.Identity,
                                 bias=binc[:, 0:1])
            hT.append(ht)

        # ---- rounds ----
        for t in range(NT):
            for b in range(BPC):
                ht = hT[b]
                w1a = we1a[:, bass.ts(t, HE)]
                w1b = we1b[:, bass.ts(t, HE)]
                w2r = we2[:, bass.ts(t, EO)]

                # receivers: biasT = rcvT + be1 ; mB = -biasT
                rcvp = ppsum.tile([HE, N], F32, tag="prep")
                nc.tensor.matmul(rcvp[:, :], w1a, ht[:, :],
                                 start=True, stop=True, skip_group_check=True)
                biasT = perb.tile([HE, N], F32, tag="biasT")
                nc.scalar.activation(biasT[:, :], rcvp[:, :], AF.Identity,
                                     bias=be1[:, t:t + 1])
                mB = perb.tile([HE, N], F32, tag="mB")
                nc.vector.tensor_scalar_mul(mB[:, :], biasT[:, :], -1.0)

                # senders: snd2 = [sndT | sndT] fp16 ; ssum = sum_j snd_j
                sndp = ppsum.tile([HE, N], F32, tag="prep")
                nc.tensor.matmul(sndp[:, :], w1b, ht[:, :],
                                 start=True, stop=True, skip_group_check=True)
                snd2 = perb.tile([HE, 2 * N], F16, tag="snd2")
                ssum = small.tile([HE, 1], F32, tag="ssum")
                nc.scalar.activation(snd2[:, 0:N], sndp[:, :], AF.Copy,
                                     accum_out=ssum[:, 0:1])
                nc.vector.tensor_copy(snd2[:, N:2 * N], snd2[:, 0:N])

                # Gamma = We2^T biasT ; C_pairs (stacked) = Gamma + be2
                gp = ppsum.tile([EO, N], F32, tag="prep")
                nc.tensor.matmul(gp[:, :], w2r, biasT[:, :],
                                 start=True, stop=True, skip_group_check=True)
                gT = perb.tile([EO, N], F32, tag="gT")
                nc.vector.tensor_copy(gT[:, :], gp[:, :])
                gv = gT[:, :].rearrange("p (n two) -> p two n", two=2)
                cpairs = perb.tile([HE, NP2], F32, tag="cpairs")
                nc.scalar.activation(cpairs[0:EO, :], gv[:, 0:1, :], AF.Identity,
                                     bias=be2[:, t:t + 1])
                nc.scalar.activation(cpairs[EO:HE, :], gv[:, 1:2, :], AF.Identity,
                                     bias=be2[:, t:t + 1])

                # K0 = 0.04 * We2^T ssum + 51.2 * be2
                sdp = ppsum.tile([EO, 1], F32, tag="prep")
                nc.tensor.matmul(sdp[:, :], w2r, ssum[:, 0:1],
                                 start=True, stop=True, skip_group_check=True)
                k0 = small.tile([EO, 1], F32, tag="k0")
                nc.scalar.activation(k0[:, :], sdp[:, :], AF.Identity,
                                     scale=0.04, bias=be2_512[t][:, 0:1])

                SM = perb.tile([HE, N], F32, tag="SM")
                mrstage = perb.tile([HE, NP2], F32, tag="mrstage")

                # ---- edge loop: 64 psum banks, 4 receivers each ----
                nbank = N // 4
                for g0 in range(0, nbank, SG):
                    banks = range(g0, min(g0 + SG, nbank))
                    pbs = {}
                    mts = {}
                    for k in banks:
                        pbs[k] = psum.tile([HE, 2 * N], F32, tag="pb",
                                           name=f"pb_{t}_{b}_{k}")
                        for q in range(4):
                            i = 4 * k + q
                            m = mpool.tile([HE, N], F16, tag="M")
                            nc.vector.tensor_scalar(
                                m[:, :], snd2[:, 0:N], mB[:, i:i + 1], None,
                                ALU.max, ALU.add, accum_out=SM[:, i:i + 1])
                            mts[(k, q)] = m
                    for k in banks:
                        nc.tensor.matmul(pbs[k][:, :], wdd[t][:, :],
                                         snd2[:, :], start=True, stop=False,
                                         skip_group_check=True)
                    for k in banks:
                        for q in range(4):
                            m = mts[(k, q)]
                            rows = slice(0, EO) if q % 2 == 0 else slice(EO, HE)
                            cols = slice(0, N) if q < 2 else slice(N, 2 * N)
                            nc.tensor.matmul(pbs[k][rows, cols], w8[t][:, :],
                                             m[:, :], start=False, stop=True,
                                             skip_group_check=True)
                    for k in banks:
                        for hh in range(2):
                            p = 2 * k + hh
                            cols = slice(0, N) if hh == 0 else slice(N, 2 * N)
                            nc.scalar.activation(
                                pbs[k][:, cols], pbs[k][:, cols], AF.Relu,
                                bias=cpairs[:, p:p + 1],
                                accum_out=mrstage[:, p:p + 1])

                # ---- batched corrections and message assembly ----
                qp = ppsum.tile([EO, N], F32, tag="prep")
                nc.tensor.matmul(qp[:, :], w2r, SM[:, :],
                                 start=True, stop=True, skip_group_check=True)
                mrT = perb.tile([EO, N], F32, tag="mrT")
                mrv = mrT[:, :].rearrange("p (n two) -> p two n", two=2)
                nc.vector.tensor_copy(mrv[:, 0:1, :], mrstage[0:EO, :])
                nc.vector.tensor_copy(mrv[:, 1:2, :], mrstage[EO:HE, :])
                u1 = perb.tile([EO, N], F32, tag="u1")
                nc.vector.scalar_tensor_tensor(u1[:, :], qp[:, :], 0.2,
                                               mrT[:, :], ALU.mult, ALU.add)
                u2 = perb.tile([EO, N], F32, tag="u2")
                nc.vector.scalar_tensor_tensor(u2[:, :], gT[:, :], 64.0,
                                               u1[:, :], ALU.mult, ALU.add)

                # n = [m; h] (Wn0 rows reordered host-side to match)
                nT = perb.tile([H + EO, N], F32, tag="nT")
                nc.scalar.activation(nT[0:EO, :], u2[:, :], AF.Identity,
                                     scale=0.8, bias=k0[:, 0:1])
                nc.scalar.copy(nT[EO:EO + H, :], ht[:, :])

                # ---- node MLP (exact lrelu via max(z, 0.2 z)) ----
                n1p = ppsum.tile([H, N], F32, tag="prep")
                nc.tensor.matmul(n1p[:, :], wn0[:, bass.ts(t, H)],
                                 nT[:, :],
                                 start=True, stop=True, skip_group_check=True)
                z1 = small.tile([H, N], F32, tag="z1")
                nc.scalar.activation(z1[:, :], n1p[:, :], AF.Identity,
                                     bias=bn0[:, t:t + 1])
                a1 = small.tile([H, N], F32, tag="a1")
                nc.vector.scalar_tensor_tensor(a1[:, :], z1[:, :], 0.2,
                                               z1[:, :], ALU.mult, ALU.max)

                n2p = ppsum.tile([H, N], F32, tag="prep")
                nc.tensor.matmul(n2p[:, :], wn1[:, bass.ts(t, H)],
                                 a1[:, :],
                                 start=True, stop=True, skip_group_check=True)
                z2 = small.tile([H, N], F32, tag="z2")
                nc.scalar.activation(z2[:, :], n2p[:, :], AF.Identity,
                                     bias=bn1[:, t:t + 1])
                if t < NT - 1:
                    ht2 = perb.tile([H, N], F32, tag=f"hT{b}")
                    nc.vector.scalar_tensor_tensor(ht2[:, :], z2[:, :], 0.2,
                                                   z2[:, :], ALU.mult, ALU.max)
                    hT[b] = ht2
                else:
                    hfin = small.tile([H, N], F32, tag="hfin")
                    nc.vector.scalar_tensor_tensor(hfin[:, :], z2[:, :], 0.2,
                                                   z2[:, :], ALU.mult, ALU.max)
                    outT = small.tile([H, N], F32, tag="outT")
                    nc.scalar.activation(outT[:, :], hfin[:, :], AF.Tanh)
                    nc.sync.dma_start(out_d[b, :, :], outT[:, :])

    nc.compile()
    return nc


def _prepare_in_maps(inputs):
    f32 = lambda a: np.ascontiguousarray(np.asarray(a), dtype=np.float32)
    x = f32(inputs["x"])
    W_lin = f32(inputs["W_lin"])
    b_lin = f32(inputs["b_lin"])
    W_in = f32(inputs["W_in"])
    b_in = f32(inputs["b_in"])
    We1 = f32(inputs["We1"])
    be1 = f32(inputs["be1"])
    We2 = f32(inputs["We2"])
    be2 = f32(inputs["be2"])
    Wn0 = f32(inputs["Wn0"])
    bn0 = f32(inputs["bn0"])
    Wn1 = f32(inputs["Wn1"])
    bn1 = f32(inputs["bn1"])

    blT = b_lin.reshape(N, L).T                     # [L, N]
    blT_perm = np.ascontiguousarray(blT[:, PERM])
    shared = {
        "Wlin": W_lin,
        "blT": blT_perm,
        "Win": W_in,
        "binc": np.ascontiguousarray(b_in[:, None]),
        "We1a": np.ascontiguousarray(We1[:, :H, :].transpose(1, 0, 2).reshape(H, NT * HE)),
        "We1b": np.ascontiguousarray(We1[:, H:, :].transpose(1, 0, 2).reshape(H, NT * HE)),
        "be1c": np.ascontiguousarray(be1.T),
        "We2c": np.ascontiguousarray(We2.transpose(1, 0, 2).reshape(HE, NT * EO)),
        "be2c": np.ascontiguousarray(be2.T),
        # n is assembled as [m; h] on-chip, so reorder Wn0's input rows
        "Wn0c": np.ascontiguousarray(
            np.concatenate([Wn0[:, H:, :], Wn0[:, :H, :]], axis=1)
            .transpose(1, 0, 2).reshape(H + EO, NT * H)),
        "bn0c": np.ascontiguousarray(bn0.T),
        "Wn1c": np.ascontiguousarray(Wn1.transpose(1, 0, 2).reshape(H, NT * H)),
        "bn1c": np.ascontiguousarray(bn1.T),
    }
    in_maps = []
    for c in range(NCORES):
        m = dict(shared)
        m["xT"] = np.ascontiguousarray(x[c * BPC:(c + 1) * BPC, :].T)
        in_maps.append(m)
    return in_maps


def _run(inputs, trace=False):
    from concourse import bass_utils

    if "nc" not in _CACHE:
        _CACHE["nc"] = _build_nc()
    nc = _CACHE["nc"]
    in_maps = _prepare_in_maps(inputs)
    res = bass_utils.run_bass_kernel_spmd(
        nc, in_maps, core_ids=list(range(NCORES)), trace=trace)
    outs = np.concatenate([r["out"] for r in res.results], axis=0)  # [B,H,N]
    out = outs[:, :, INVPERM]            # undo node permutation
    out = np.ascontiguousarray(out.transpose(0, 2, 1))  # [B, N, H]
    return out.astype(np.float32), res


def kernel(**inputs):
    out, _ = _run(inputs, trace=False)
    return out


# revision 32
# speedup vs baseline: 1.1643x; 1.1643x over previous
"""Trainium2 Bass kernel for the GNN message-passing decoder.

Model (per batch b):
  h0 = x @ W_lin + b_lin            -> [N=256, L2=64] per b
  h  = h0 @ W_in + b_in             -> [N, H=32]
  3 rounds of fully-connected message passing:
    rcv = h @ We1[:H], snd = h @ We1[H:]
    e1  = lrelu(rcv_i + snd_j + be1)          [N,N,HE=128]
    e2  = lrelu(e1 @ We2 + be2)               [N,N,EO=64]
    m_i = sum_j e2                            [N,EO]
    n   = lrelu([h|m] @ Wn0 + bn0); h = lrelu(n @ Wn1 + bn1)
  out = tanh(h)                      -> [B, N, H]

Strategy: pure data parallel over batch (16 -> 2 per core on 8 cores).
Per (b, round): feature-on-partition layout. For each receiver i the DVE
computes M_i = max(sndT, -b_i) in fp16 (4x mode) with a free-dim running sum
(accum) in the same instruction; since relu(z) = M_i + b_i and
lrelu(z) = 0.2 z + 0.8 relu(z), the PE accumulates
P = 0.8 We2^T M_i + 0.2 We2^T sndT into PSUM (two receivers partition-stacked
per half-bank) and the remaining per-receiver constant We2^T b_i + be2 becomes
the per-partition bias of the ACT layer-2 relu, whose accum output yields
sum_j relu(g) directly. The linear halves of both lrelus are restored exactly
with per-(b,t) batched corrections (Q = We2^T sum_j M, ssum terms).
"""

import os
import sys

import numpy as np

for _p in ("/opt/trn_rl_repo", "/opt/pypackages"):
    if _p not in sys.path and os.path.isdir(_p):
        sys.path.append(_p)

# Problem dims (hardcoded per spec)
B, N, L, H, HE, EO = 16, 256, 64, 32, 128, 64
NT = 3           # message passing rounds
NCORES = 8
BPC = B // NCORES  # batches per core = 2
NP2 = N // 2     # 128 n-pair matmuls in stage 1
SG = 3           # psum banks per supergroup (LDW amortization)

# column permutation of the node axis used on-chip (even nodes first)
PERM = np.concatenate([np.arange(0, N, 2), np.arange(1, N, 2)])
INVPERM = np.argsort(PERM)

_CACHE = {}


def _build_nc():
    import concourse.bass as bass
    import concourse.tile as tile
    from concourse import bacc, mybir
    from contextlib import ExitStack

    F16 = mybir.dt.float16
    F32 = mybir.dt.float32
    F32R = mybir.dt.float32r
    AF = mybir.ActivationFunctionType
    ALU = mybir.AluOpType

    nc = bacc.Bacc("TRN2", target_bir_lowering=False, debug=False)

    # ---- kernel I/O (per-core) ----
    xT_d = nc.dram_tensor("xT", [L, BPC], F32, kind="ExternalInput")
    wlin_d = nc.dram_tensor("Wlin", [L, N * L], F32, kind="ExternalInput")
    blT_d = nc.dram_tensor("blT", [L, N], F32, kind="ExternalInput")  # perm'd
    win_d = nc.dram_tensor("Win", [L, H], F32, kind="ExternalInput")
    binc_d = nc.dram_tensor("binc", [H, 1], F32, kind="ExternalInput")
    we1a_d = nc.dram_tensor("We1a", [H, NT * HE], F32, kind="ExternalInput")
    we1b_d = nc.dram_tensor("We1b", [H, NT * HE], F32, kind="ExternalInput")
    be1_d = nc.dram_tensor("be1c", [HE, NT], F32, kind="ExternalInput")
    we2_d = nc.dram_tensor("We2c", [HE, NT * EO], F32, kind="ExternalInput")
    be2_d = nc.dram_tensor("be2c", [EO, NT], F32, kind="ExternalInput")
    wn0_d = nc.dram_tensor("Wn0c", [H + EO, NT * H], F32, kind="ExternalInput")
    bn0_d = nc.dram_tensor("bn0c", [H, NT], F32, kind="ExternalInput")
    wn1_d = nc.dram_tensor("Wn1c", [H, NT * H], F32, kind="ExternalInput")
    bn1_d = nc.dram_tensor("bn1c", [H, NT], F32, kind="ExternalInput")
    out_d = nc.dram_tensor("out", [BPC, H, N], F32, kind="ExternalOutput")

    with tile.TileContext(nc) as tc, ExitStack() as ctx:
        const = ctx.enter_context(tc.tile_pool(name="const", bufs=1))
        perb = ctx.enter_context(tc.tile_pool(name="perb", bufs=2))
        mpool = ctx.enter_context(tc.tile_pool(name="m", bufs=10))
        e2pool = ctx.enter_context(tc.tile_pool(name="e2p", bufs=3))
        small = ctx.enter_context(tc.tile_pool(name="small", bufs=4))
        psum = ctx.enter_context(tc.tile_pool(name="psum", bufs=3, space="PSUM"))
        ppsum = ctx.enter_context(tc.tile_pool(name="ppsum", bufs=2, space="PSUM"))

        # ---- load constants ----
        def load(dram, shape):
            t = const.tile(shape, F32, tag=dram.name)
            nc.sync.dma_start(t[:, :], dram[:, :])
            return t

        xTs = load(xT_d, [L, BPC])
        blT = load(blT_d, [L, N])
        win = load(win_d, [L, H])
        binc = load(binc_d, [H, 1])
        we1a = load(we1a_d, [H, NT * HE])
        we1b = load(we1b_d, [H, NT * HE])
        be1 = load(be1_d, [HE, NT])
        we2 = load(we2_d, [HE, NT * EO])
        be2 = load(be2_d, [EO, NT])
        wn0 = load(wn0_d, [H + EO, NT * H])
        bn0 = load(bn0_d, [H, NT])
        wn1 = load(wn1_d, [H, NT * H])
        bn1 = load(bn1_d, [H, NT])

        wlin = const.tile([L, N * L], F32, tag="wlin")
        for k in range(4):
            sl = bass.ts(k, N * L // 4)
            eng = nc.sync if k % 2 == 0 else nc.gpsimd
            eng.dma_start(wlin[:, sl], wlin_d[:, sl])

        # fp16 / scaled weight variants per round
        w8 = []    # 0.8 * We2, fp16 [HE, EO]
        wdd = []   # 0.2 * We2 duplicated, fp16 [HE, 2*EO]
        for t in range(NT):
            w2sl = we2[:, bass.ts(t, EO)]
            a = const.tile([HE, EO], F16, tag=f"w8_{t}")
            nc.scalar.mul(a[:, :], w2sl, 0.8)
            w8.append(a)
            d = const.tile([HE, 2 * EO], F16, tag=f"wdd_{t}")
            nc.scalar.mul(d[:, 0:EO], w2sl, 0.2)
            nc.scalar.mul(d[:, EO:2 * EO], w2sl, 0.2)
            wdd.append(d)

        # ---- stage 1: h0 = x @ W_lin in transposed (perm'd) layout ----
        h0p = ppsum.tile([128, 2 * NP2], F32, tag="prep")
        for np_ in range(NP2):
            lhsT = wlin[:, np_ * 2 * L:(np_ + 1) * 2 * L]
            nc.tensor.matmul(h0p[:, 2 * np_:2 * np_ + 2], lhsT,
                             xTs[:, :],
                             start=True, stop=True, skip_group_check=True)
        hstA = const.tile([L, 2 * NP2], F32, tag="hstA")  # even nodes
        hstB = const.tile([L, 2 * NP2], F32, tag="hstB")  # odd nodes
        nc.scalar.copy(hstA[:, :], h0p[0:L, :])
        nc.scalar.copy(hstB[:, :], h0p[L:2 * L, :])
        # columns are (np, b) interleaved; view as [p, b, np] to slice per-b
        hsvA = hstA[:, :].rearrange("p (n two) -> p two n", two=2)
        hsvB = hstB[:, :].rearrange("p (n two) -> p two n", two=2)

        hT = []  # per-b current hidden state tiles [H, N] f32
        for b in range(BPC):
            htp = ppsum.tile([H, N], F32, tag="prep")
            # full-width start=True first; partition/free-subset accumulates after
            nc.tensor.matmul(htp[:, :], win[:, :],
                             blT[:, :],
                             start=True, stop=False, skip_group_check=True)
            nc.tensor.matmul(htp[:, 0:NP2], win[:, :],
                             hsvA[:, b:b + 1, :],
                             start=False, stop=True, skip_group_check=True)
            nc.tensor.matmul(htp[:, NP2:N], win[:, :],
                             hsvB[:, b:b + 1, :],
                             start=False, stop=True, skip_group_check=True)
            ht = perb.tile([H, N], F32, tag=f"hT{b}")
            nc.scalar.activation(ht[:, :], htp[:, :], AF.Identity,
                                 bias=binc[:, 0:1])
            hT.append(ht)

        # ---- rounds ----
        for t in range(NT):
            for b in range(BPC):
                ht = hT[b]
                w1a = we1a[:, bass.ts(t, HE)]
                w1b = we1b[:, bass.ts(t, HE)]
                w2r = we2[:, bass.ts(t, EO)]

                # receivers: biasT = rcvT + be1
                rcvp = ppsum.tile([HE, N], F32, tag="prep")
                nc.tensor.matmul(rcvp[:, :], w1a, ht[:, :],
                                 start=True, stop=True, skip_group_check=True)
                biasT = perb.tile([HE, N], F32, tag="biasT")
                nc.scalar.activation(biasT[:, :], rcvp[:, :], AF.Identity,
                                     bias=be1[:, t:t + 1])

                # senders: snd2 = [sndT | sndT] fp16
                sndp = ppsum.tile([HE, N], F32, tag="prep")
                nc.tensor.matmul(sndp[:, :], w1b, ht[:, :],
                                 start=True, stop=True, skip_group_check=True)
                snd2 = perb.tile([HE, 2 * N], F16, tag="snd2")
                nc.scalar.activation(snd2[:, 0:N], sndp[:, :], AF.Copy)
                nc.vector.tensor_copy(snd2[:, N:2 * N], snd2[:, 0:N])

                # c_pairs = 0.2 * We2^T biasT + be2 (stacked even/odd)
                gp = ppsum.tile([EO, N], F32, tag="prep")
                nc.tensor.matmul(gp[:, :], w2r, biasT[:, :],
                                 start=True, stop=True, skip_group_check=True)
                gv = gp[:, :].rearrange("p (n two) -> p two n", two=2)
                cpairs = perb.tile([HE, NP2], F32, tag="cpairs")
                nc.scalar.activation(cpairs[0:EO, :], gv[:, 0:1, :],
                                     AF.Identity, scale=0.2,
                                     bias=be2[:, t:t + 1])
                nc.scalar.activation(cpairs[EO:HE, :], gv[:, 1:2, :],
                                     AF.Identity, scale=0.2,
                                     bias=be2[:, t:t + 1])

                mrstage = perb.tile([HE, NP2], F32, tag="mrstage")

                # ---- edge loop: 32 double-bank psum tiles, 8 receivers each
                # R_i = relu(sndT + b_i); PSUM = 0.8 We2^T R_i + d;
                # L2: e2 = prelu(P + c_pair) -> fp16 arena; sums via DVE reduce
                ndbl = N // 8
                for g0 in range(0, ndbl, SG):
                    dbls = range(g0, min(g0 + SG, ndbl))
                    pbs = {}
                    mts = {}
                    for k in dbls:
                        pbs[k] = psum.tile([HE, 4 * N], F32, tag="pb",
                                           name=f"pb_{t}_{b}_{k}")
                        for q in range(8):
                            i = 8 * k + q
                            m = mpool.tile([HE, N], F16, tag="M")
                            nc.vector.tensor_scalar(
                                m[:, :], snd2[:, 0:N], biasT[:, i:i + 1],
                                0.0, ALU.add, ALU.max)
                            mts[(k, q)] = m
                    for k in dbls:
                        nc.tensor.matmul(pbs[k][:, 0:2 * N], wdd[t][:, :],
                                         snd2[:, :], start=True, stop=False,
                                         skip_group_check=True)
                        nc.tensor.matmul(pbs[k][:, 2 * N:4 * N], wdd[t][:, :],
                                         snd2[:, :], start=True, stop=False,
                                         skip_group_check=True)
                    for k in dbls:
                        for q in range(8):
                            m = mts[(k, q)]
                            rows = slice(0, EO) if q % 2 == 0 else slice(EO, HE)
                            cols = slice((q // 2) * N, (q // 2 + 1) * N)
                            nc.tensor.matmul(pbs[k][rows, cols], w8[t][:, :],
                                             m[:, :], start=False, stop=True,
                                             skip_group_check=True)
                    for k in dbls:
                        e2 = e2pool.tile([HE, 4 * N], F16, tag="e2",
                                         name=f"e2_{t}_{b}_{k}")
                        for c in range(4):
                            p = 4 * k + c
                            cs = slice(c * N, (c + 1) * N)
                            nc.scalar.activation(e2[:, cs], pbs[k][:, cs],
                                                 AF.Prelu, alpha=0.2,
                                                 bias=cpairs[:, p:p + 1])
                        e2v = e2[:, :].rearrange("p (four n) -> p four n",
                                                 four=4)
                        nc.vector.tensor_reduce(
                            mrstage[:, 4 * k:4 * k + 4], e2v[:, :, :],
                            axis=mybir.AxisListType.X, op=ALU.add)

                # ---- message assembly: n = [m; h] (Wn0 rows reordered) ----
                nT = perb.tile([H + EO, N], F32, tag="nT")
                nv = nT[0:EO, :].rearrange("p (n two) -> p two n", two=2)
                nc.vector.tensor_copy(nv[:, 0:1, :], mrstage[0:EO, :])
                nc.vector.tensor_copy(nv[:, 1:2, :], mrstage[EO:HE, :])
                nc.scalar.copy(nT[EO:EO + H, :], ht[:, :])

                # ---- node MLP (exact lrelu via max(z, 0.2 z)) ----
                n1p = ppsum.tile([H, N], F32, tag="prep")
                nc.tensor.matmul(n1p[:, :], wn0[:, bass.ts(t, H)],
                                 nT[:, :],
                                 start=True, stop=True, skip_group_check=True)
                z1 = small.tile([H, N], F32, tag="z1")
                nc.scalar.activation(z1[:, :], n1p[:, :], AF.Identity,
                                     bias=bn0[:, t:t + 1])
                a1 = small.tile([H, N], F32, tag="a1")
                nc.vector.scalar_tensor_tensor(a1[:, :], z1[:, :], 0.2,
                                               z1[:, :], ALU.mult, ALU.max)

                n2p = ppsum.tile([H, N], F32, tag="prep")
                nc.tensor.matmul(n2p[:, :], wn1[:, bass.ts(t, H)],
                                 a1[:, :],
                                 start=True, stop=True, skip_group_check=True)
                z2 = small.tile([H, N], F32, tag="z2")
                nc.scalar.activation(z2[:, :], n2p[:, :], AF.Identity,
                                     bias=bn1[:, t:t + 1])
                if t < NT - 1:
                    ht2 = perb.tile([H, N], F32, tag=f"hT{b}")
                    nc.vector.scalar_tensor_tensor(ht2[:, :], z2[:, :], 0.2,
                                                   z2[:, :], ALU.mult, ALU.max)
                    hT[b] = ht2
                else:
                    hfin = small.tile([H, N], F32, tag="hfin")
                    nc.vector.scalar_tensor_tensor(hfin[:, :], z2[:, :], 0.2,
                                                   z2[:, :], ALU.mult, ALU.max)
                    outT = small.tile([H, N], F32, tag="outT")
                    nc.scalar.activation(outT[:, :], hfin[:, :], AF.Tanh)
                    nc.sync.dma_start(out_d[b, :, :], outT[:, :])

    nc.compile()
    return nc


def _prepare_in_maps(inputs):
    f32 = lambda a: np.ascontiguousarray(np.asarray(a), dtype=np.float32)
    x = f32(inputs["x"])
    W_lin = f32(inputs["W_lin"])
    b_lin = f32(inputs["b_lin"])
    W_in = f32(inputs["W_in"])
    b_in = f32(inputs["b_in"])
    We1 = f32(inputs["We1"])
    be1 = f32(inputs["be1"])
    We2 = f32(inputs["We2"])
    be2 = f32(inputs["be2"])
    Wn0 = f32(inputs["Wn0"])
    bn0 = f32(inputs["bn0"])
    Wn1 = f32(inputs["Wn1"])
    bn1 = f32(inputs["bn1"])

    blT = b_lin.reshape(N, L).T                     # [L, N]
    blT_perm = np.ascontiguousarray(blT[:, PERM])
    shared = {
        "Wlin": W_lin,
        "blT": blT_perm,
        "Win": W_in,
        "binc": np.ascontiguousarray(b_in[:, None]),
        "We1a": np.ascontiguousarray(We1[:, :H, :].transpose(1, 0, 2).reshape(H, NT * HE)),
        "We1b": np.ascontiguousarray(We1[:, H:, :].transpose(1, 0, 2).reshape(H, NT * HE)),
        "be1c": np.ascontiguousarray(be1.T),
        "We2c": np.ascontiguousarray(We2.transpose(1, 0, 2).reshape(HE, NT * EO)),
        "be2c": np.ascontiguousarray(be2.T),
        # n is assembled as [m; h] on-chip, so reorder Wn0's input rows
        "Wn0c": np.ascontiguousarray(
            np.concatenate([Wn0[:, H:, :], Wn0[:, :H, :]], axis=1)
            .transpose(1, 0, 2).reshape(H + EO, NT * H)),
        "bn0c": np.ascontiguousarray(bn0.T),
        "Wn1c": np.ascontiguousarray(Wn1.transpose(1, 0, 2).reshape(H, NT * H)),
        "bn1c": np.ascontiguousarray(bn1.T),
    }
    in_maps = []
    for c in range(NCORES):
        m = dict(shared)
        m["xT"] = np.ascontiguousarray(x[c * BPC:(c + 1) * BPC, :].T)
        in_maps.append(m)
    return in_maps


def _run(inputs, trace=False):
    from concourse import bass_utils

    if "nc" not in _CACHE:
        _CACHE["nc"] = _build_nc()
    nc = _CACHE["nc"]
    in_maps = _prepare_in_maps(inputs)
    res = bass_utils.run_bass_kernel_spmd(
        nc, in_maps, core_ids=list(range(NCORES)), trace=trace)
    outs = np.concatenate([r["out"] for r in res.results], axis=0)  # [B,H,N]
    out = outs[:, :, INVPERM]            # undo node permutation
    out = np.ascontiguousarray(out.transpose(0, 2, 1))  # [B, N, H]
    return out.astype(np.float32), res


def kernel(**inputs):
    out, _ = _run(inputs, trace=False)
    return out


# revision 37
# speedup vs baseline: 1.3035x; 1.1196x over previous
"""Trainium2 Bass kernel for the GNN message-passing decoder.

Model (per batch b):
  h0 = x @ W_lin + b_lin            -> [N=256, L2=64] per b
  h  = h0 @ W_in + b_in             -> [N, H=32]
  3 rounds of fully-connected message passing:
    rcv = h @ We1[:H], snd = h @ We1[H:]
    e1  = lrelu(rcv_i + snd_j + be1)          [N,N,HE=128]
    e2  = lrelu(e1 @ We2 + be2)               [N,N,EO=64]
    m_i = sum_j e2                            [N,EO]
    n   = lrelu([h|m] @ Wn0 + bn0); h = lrelu(n @ Wn1 + bn1)
  out = tanh(h)                      -> [B, N, H]

Strategy: pure data parallel over batch (16 -> 2 per core on 8 cores).
Per (b, round): feature-on-partition layout. For each receiver i the DVE
computes M_i = max(sndT, -b_i) in fp16 (4x mode) with a free-dim running sum
(accum) in the same instruction; since relu(z) = M_i + b_i and
lrelu(z) = 0.2 z + 0.8 relu(z), the PE accumulates
P = 0.8 We2^T M_i + 0.2 We2^T sndT into PSUM (two receivers partition-stacked
per half-bank) and the remaining per-receiver constant We2^T b_i + be2 becomes
the per-partition bias of the ACT layer-2 relu, whose accum output yields
sum_j relu(g) directly. The linear halves of both lrelus are restored exactly
with per-(b,t) batched corrections (Q = We2^T sum_j M, ssum terms).
"""

import os
import sys

import numpy as np

for _p in ("/opt/trn_rl_repo", "/opt/pypackages"):
    if _p not in sys.path and os.path.isdir(_p):
        sys.path.append(_p)

# Problem dims (hardcoded per spec)
B, N, L, H, HE, EO = 16, 256, 64, 32, 128, 64
NT = 3           # message passing rounds
NCORES = 8
BPC = B // NCORES  # batches per core = 2
NP2 = N // 2     # 128 n-pair matmuls in stage 1
SG = 3           # psum banks per supergroup (LDW amortization)

# column permutation of the node axis used on-chip (even nodes first)
PERM = np.concatenate([np.arange(0, N, 2), np.arange(1, N, 2)])
INVPERM = np.argsort(PERM)

_CACHE = {}


def _build_nc():
    import concourse.bass as bass
    import concourse.tile as tile
    from concourse import bacc, mybir
    from contextlib import ExitStack

    F16 = mybir.dt.float16
    F32 = mybir.dt.float32
    F32R = mybir.dt.float32r
    AF = mybir.ActivationFunctionType
    ALU = mybir.AluOpType

    nc = bacc.Bacc("TRN2", target_bir_lowering=False, debug=False)

    # ---- kernel I/O (per-core) ----
    xT_d = nc.dram_tensor("xT", [L, BPC], F32, kind="ExternalInput")
    wlin_d = nc.dram_tensor("Wlin", [L, N * L], F32, kind="ExternalInput")
    blT_d = nc.dram_tensor("blT", [L, N], F32, kind="ExternalInput")  # perm'd
    win_d = nc.dram_tensor("Win", [L, H], F32, kind="ExternalInput")
    binc_d = nc.dram_tensor("binc", [H, 1], F32, kind="ExternalInput")
    we1a_d = nc.dram_tensor("We1a", [H, NT * HE], F32, kind="ExternalInput")
    we1b_d = nc.dram_tensor("We1b", [H, NT * HE], F32, kind="ExternalInput")
    be1_d = nc.dram_tensor("be1c", [HE, NT], F32, kind="ExternalInput")
    we2_d = nc.dram_tensor("We2c", [HE, NT * EO], F32, kind="ExternalInput")
    be2_d = nc.dram_tensor("be2c", [EO, NT], F32, kind="ExternalInput")
    wn0_d = nc.dram_tensor("Wn0c", [H + EO, NT * H], F32, kind="ExternalInput")
    bn0_d = nc.dram_tensor("bn0c", [H, NT], F32, kind="ExternalInput")
    wn1_d = nc.dram_tensor("Wn1c", [H, NT * H], F32, kind="ExternalInput")
    bn1_d = nc.dram_tensor("bn1c", [H, NT], F32, kind="ExternalInput")
    out_d = nc.dram_tensor("out", [BPC, H, N], F32, kind="ExternalOutput")

    with tile.TileContext(nc) as tc, ExitStack() as ctx:
        const = ctx.enter_context(tc.tile_pool(name="const", bufs=1))
        perb = ctx.enter_context(tc.tile_pool(name="perb", bufs=2))
        mpool = ctx.enter_context(tc.tile_pool(name="m", bufs=10))
        e2pool = ctx.enter_context(tc.tile_pool(name="e2p", bufs=3))
        small = ctx.enter_context(tc.tile_pool(name="small", bufs=4))
        psum = ctx.enter_context(tc.tile_pool(name="psum", bufs=3, space="PSUM"))
        ppsum = ctx.enter_context(tc.tile_pool(name="ppsum", bufs=2, space="PSUM"))

        # ---- load constants ----
        def load(dram, shape):
            t = const.tile(shape, F32, tag=dram.name)
            nc.sync.dma_start(t[:, :], dram[:, :])
            return t

        xTs = load(xT_d, [L, BPC])
        blT = load(blT_d, [L, N])
        win = load(win_d, [L, H])
        binc = load(binc_d, [H, 1])
        we1a = load(we1a_d, [H, NT * HE])
        we1b = load(we1b_d, [H, NT * HE])
        be1 = load(be1_d, [HE, NT])
        we2 = load(we2_d, [HE, NT * EO])
        be2 = load(be2_d, [EO, NT])
        wn0 = load(wn0_d, [H + EO, NT * H])
        bn0 = load(bn0_d, [H, NT])
        wn1 = load(wn1_d, [H, NT * H])
        bn1 = load(bn1_d, [H, NT])

        wlin = const.tile([L, N * L], F32, tag="wlin")
        for k in range(4):
            sl = bass.ts(k, N * L // 4)
            eng = nc.sync if k % 2 == 0 else nc.gpsimd
            eng.dma_start(wlin[:, sl], wlin_d[:, sl])

        # fp16 / scaled weight variants per round
        w8 = []    # 0.8 * We2, fp16 [HE, EO]
        wdd = []   # 0.2 * We2 duplicated, fp16 [HE, 2*EO]
        for t in range(NT):
            w2sl = we2[:, bass.ts(t, EO)]
            a = const.tile([HE, EO], F16, tag=f"w8_{t}")
            nc.scalar.mul(a[:, :], w2sl, 0.8)
            w8.append(a)
            d = const.tile([HE, 2 * EO], F16, tag=f"wdd_{t}")
            nc.scalar.mul(d[:, 0:EO], w2sl, 0.2)
            nc.scalar.mul(d[:, EO:2 * EO], w2sl, 0.2)
            wdd.append(d)

        # ---- stage 1: h0 = x @ W_lin in transposed (perm'd) layout ----
        h0p = ppsum.tile([128, 2 * NP2], F32, tag="prep")
        for np_ in range(NP2):
            lhsT = wlin[:, np_ * 2 * L:(np_ + 1) * 2 * L]
            nc.tensor.matmul(h0p[:, 2 * np_:2 * np_ + 2], lhsT,
                             xTs[:, :],
                             start=True, stop=True, skip_group_check=True)
        hstA = const.tile([L, 2 * NP2], F32, tag="hstA")  # even nodes
        hstB = const.tile([L, 2 * NP2], F32, tag="hstB")  # odd nodes
        nc.scalar.copy(hstA[:, :], h0p[0:L, :])
        nc.scalar.copy(hstB[:, :], h0p[L:2 * L, :])
        # columns are (np, b) interleaved; view as [p, b, np] to slice per-b
        hsvA = hstA[:, :].rearrange("p (n two) -> p two n", two=2)
        hsvB = hstB[:, :].rearrange("p (n two) -> p two n", two=2)

        hT = []  # per-b current hidden state tiles [H, N] f32
        for b in range(BPC):
            htp = ppsum.tile([H, N], F32, tag="prep")
            # full-width start=True first; partition/free-subset accumulates after
            nc.tensor.matmul(htp[:, :], win[:, :],
                             blT[:, :],
                             start=True, stop=False, skip_group_check=True)
            nc.tensor.matmul(htp[:, 0:NP2], win[:, :],
                             hsvA[:, b:b + 1, :],
                             start=False, stop=True, skip_group_check=True)
            nc.tensor.matmul(htp[:, NP2:N], win[:, :],
                             hsvB[:, b:b + 1, :],
                             start=False, stop=True, skip_group_check=True)
            ht = perb.tile([H, N], F32, tag=f"hT{b}")
            nc.scalar.activation(ht[:, :], htp[:, :], AF.Identity,
                                 bias=binc[:, 0:1])
            hT.append(ht)

        # ---- rounds ----
        for t in range(NT):
            for b in range(BPC):
                ht = hT[b]
                w1a = we1a[:, bass.ts(t, HE)]
                w1b = we1b[:, bass.ts(t, HE)]
                w2r = we2[:, bass.ts(t, EO)]

                # receivers: biasT = rcvT + be1
                rcvp = ppsum.tile([HE, N], F32, tag="prep")
                nc.tensor.matmul(rcvp[:, :], w1a, ht[:, :],
                                 start=True, stop=True, skip_group_check=True)
                biasT = perb.tile([HE, N], F32, tag="biasT")
                nc.scalar.activation(biasT[:, :], rcvp[:, :], AF.Identity,
                                     bias=be1[:, t:t + 1])

                # senders: snd2 = [sndT | sndT] fp16
                sndp = ppsum.tile([HE, N], F32, tag="prep")
                nc.tensor.matmul(sndp[:, :], w1b, ht[:, :],
                                 start=True, stop=True, skip_group_check=True)
                snd2 = perb.tile([HE, 2 * N], F16, tag="snd2")
                nc.scalar.activation(snd2[:, 0:N], sndp[:, :], AF.Copy)
                nc.vector.tensor_copy(snd2[:, N:2 * N], snd2[:, 0:N])

                # c_pairs = 0.2 * We2^T biasT + be2 (stacked even/odd)
                gp = ppsum.tile([EO, N], F32, tag="prep")
                nc.tensor.matmul(gp[:, :], w2r, biasT[:, :],
                                 start=True, stop=True, skip_group_check=True)
                gv = gp[:, :].rearrange("p (n two) -> p two n", two=2)
                cpairs = perb.tile([HE, NP2], F32, tag="cpairs")
                nc.scalar.activation(cpairs[0:EO, :], gv[:, 0:1, :],
                                     AF.Identity, scale=0.2,
                                     bias=be2[:, t:t + 1])
                nc.scalar.activation(cpairs[EO:HE, :], gv[:, 1:2, :],
                                     AF.Identity, scale=0.2,
                                     bias=be2[:, t:t + 1])

                mr16 = perb.tile([HE, NP2], F16, tag="mr16")

                # ---- edge loop: 32 double-bank psum tiles, 8 receivers each
                # R_i = relu(sndT + b_i); PSUM = 0.8 We2^T R_i + d;
                # L2: e2 = prelu(P + c_pair) -> fp16 arena; sums via DVE reduce
                ndbl = N // 8
                for g0 in range(0, ndbl, SG):
                    dbls = range(g0, min(g0 + SG, ndbl))
                    pbs = {}
                    mts = {}
                    for k in dbls:
                        pbs[k] = psum.tile([HE, 4 * N], F32, tag="pb",
                                           name=f"pb_{t}_{b}_{k}")
                        for q in range(8):
                            i = 8 * k + q
                            m = mpool.tile([HE, N], F16, tag="M")
                            nc.vector.tensor_scalar(
                                m[:, :], snd2[:, 0:N], biasT[:, i:i + 1],
                                0.0, ALU.add, ALU.max)
                            mts[(k, q)] = m
                    for k in dbls:
                        nc.tensor.matmul(pbs[k][:, 0:2 * N], wdd[t][:, :],
                                         snd2[:, :], start=True, stop=False,
                                         skip_group_check=True)
                        nc.tensor.matmul(pbs[k][:, 2 * N:4 * N], wdd[t][:, :],
                                         snd2[:, :], start=True, stop=False,
                                         skip_group_check=True)
                    for k in dbls:
                        for q in range(8):
                            m = mts[(k, q)]
                            rows = slice(0, EO) if q % 2 == 0 else slice(EO, HE)
                            cols = slice((q // 2) * N, (q // 2 + 1) * N)
                            nc.tensor.matmul(pbs[k][rows, cols], w8[t][:, :],
                                             m[:, :], start=False, stop=True,
                                             skip_group_check=True)
                    for k in dbls:
                        if k % 3 == 2:
                            # ACT path: Prelu in-place + accumulator sums
                            with nc.allow_low_precision("m sums fp16 ok"):
                                for c in range(4):
                                    p = 4 * k + c
                                    cs = slice(c * N, (c + 1) * N)
                                    nc.scalar.activation(
                                        pbs[k][:, cs], pbs[k][:, cs], AF.Prelu,
                                        alpha=0.2, bias=cpairs[:, p:p + 1],
                                        accum_out=mr16[:, p:p + 1])
                            continue
                        e2 = e2pool.tile([HE, 4 * N], F16, tag="e2",
                                         name=f"e2_{t}_{b}_{k}")
                        for c in range(4):
                            p = 4 * k + c
                            cs = slice(c * N, (c + 1) * N)
                            nc.scalar.activation(e2[:, cs], pbs[k][:, cs],
                                                 AF.Prelu, alpha=0.2,
                                                 bias=cpairs[:, p:p + 1])
                        e2v = e2[:, :].rearrange("p (four n) -> p four n",
                                                 four=4)
                        with nc.allow_low_precision("m sums fp16 ok"):
                            nc.vector.tensor_reduce(
                                mr16[:, 4 * k:4 * k + 4], e2v[:, :, :],
                                axis=mybir.AxisListType.X, op=ALU.add)

                # ---- message assembly: n = [m; h] (Wn0 rows reordered) ----
                nT = perb.tile([H + EO, N], F32, tag="nT")
                nv = nT[0:EO, :].rearrange("p (n two) -> p two n", two=2)
                nc.vector.tensor_copy(nv[:, 0:1, :], mr16[0:EO, :])
                nc.vector.tensor_copy(nv[:, 1:2, :], mr16[EO:HE, :])
                nc.scalar.copy(nT[EO:EO + H, :], ht[:, :])

                # ---- node MLP (exact lrelu via max(z, 0.2 z)) ----
                n1p = ppsum.tile([H, N], F32, tag="prep")
                nc.tensor.matmul(n1p[:, :], wn0[:, bass.ts(t, H)],
                                 nT[:, :],
                                 start=True, stop=True, skip_group_check=True)
                z1 = small.tile([H, N], F32, tag="z1")
                nc.scalar.activation(z1[:, :], n1p[:, :], AF.Identity,
                                     bias=bn0[:, t:t + 1])
                a1 = small.tile([H, N], F32, tag="a1")
                nc.vector.scalar_tensor_tensor(a1[:, :], z1[:, :], 0.2,
                                               z1[:, :], ALU.mult, ALU.max)

                n2p = ppsum.tile([H, N], F32, tag="prep")
                nc.tensor.matmul(n2p[:, :], wn1[:, bass.ts(t, H)],
                                 a1[:, :],
                                 start=True, stop=True, skip_group_check=True)
                z2 = small.tile([H, N], F32, tag="z2")
                nc.scalar.activation(z2[:, :], n2p[:, :], AF.Identity,
                                     bias=bn1[:, t:t + 1])
                if t < NT - 1:
                    ht2 = perb.tile([H, N], F32, tag=f"hT{b}")
                    nc.vector.scalar_tensor_tensor(ht2[:, :], z2[:, :], 0.2,
                                                   z2[:, :], ALU.mult, ALU.max)
                    hT[b] = ht2
                else:
                    hfin = small.tile([H, N], F32, tag="hfin")
                    nc.vector.scalar_tensor_tensor(hfin[:, :], z2[:, :], 0.2,
                                                   z2[:, :], ALU.mult, ALU.max)
                    outT = small.tile([H, N], F32, tag="outT")
                    nc.scalar.activation(outT[:, :], hfin[:, :], AF.Tanh)
                    nc.sync.dma_start(out_d[b, :, :], outT[:, :])

    nc.compile()
    return nc


def _prepare_in_maps(inputs):
    f32 = lambda a: np.ascontiguousarray(np.asarray(a), dtype=np.float32)
    x = f32(inputs["x"])
    W_lin = f32(inputs["W_lin"])
    b_lin = f32(inputs["b_lin"])
    W_in = f32(inputs["W_in"])
    b_in = f32(inputs["b_in"])
    We1 = f32(inputs["We1"])
    be1 = f32(inputs["be1"])
    We2 = f32(inputs["We2"])
    be2 = f32(inputs["be2"])
    Wn0 = f32(inputs["Wn0"])
    bn0 = f32(inputs["bn0"])
    Wn1 = f32(inputs["Wn1"])
    bn1 = f32(inputs["bn1"])

    blT = b_lin.reshape(N, L).T                     # [L, N]
    blT_perm = np.ascontiguousarray(blT[:, PERM])
    shared = {
        "Wlin": W_lin,
        "blT": blT_perm,
        "Win": W_in,
        "binc": np.ascontiguousarray(b_in[:, None]),
        "We1a": np.ascontiguousarray(We1[:, :H, :].transpose(1, 0, 2).reshape(H, NT * HE)),
        "We1b": np.ascontiguousarray(We1[:, H:, :].transpose(1, 0, 2).reshape(H, NT * HE)),
        "be1c": np.ascontiguousarray(be1.T),
        "We2c": np.ascontiguousarray(We2.transpose(1, 0, 2).reshape(HE, NT * EO)),
        "be2c": np.ascontiguousarray(be2.T),
        # n is assembled as [m; h] on-chip, so reorder Wn0's input rows
        "Wn0c": np.ascontiguousarray(
            np.concatenate([Wn0[:, H:, :], Wn0[:, :H, :]], axis=1)
            .transpose(1, 0, 2).reshape(H + EO, NT * H)),
        "bn0c": np.ascontiguousarray(bn0.T),
        "Wn1c": np.ascontiguousarray(Wn1.transpose(1, 0, 2).reshape(H, NT * H)),
        "bn1c": np.ascontiguousarray(bn1.T),
    }
    in_maps = []
    for c in range(NCORES):
        m = dict(shared)
        m["xT"] = np.ascontiguousarray(x[c * BPC:(c + 1) * BPC, :].T)
        in_maps.append(m)
    return in_maps


def _run(inputs, trace=False):
    from concourse import bass_utils

    if "nc" not in _CACHE:
        _CACHE["nc"] = _build_nc()
    nc = _CACHE["nc"]
    in_maps = _prepare_in_maps(inputs)
    res = bass_utils.run_bass_kernel_spmd(
        nc, in_maps, core_ids=list(range(NCORES)), trace=trace)
    outs = np.concatenate([r["out"] for r in res.results], axis=0)  # [B,H,N]
    out = outs[:, :, INVPERM]            # undo node permutation
    out = np.ascontiguousarray(out.transpose(0, 2, 1))  # [B, N, H]
    return out.astype(np.float32), res


def kernel(**inputs):
    out, _ = _run(inputs, trace=False)
    return out
